# revision 20
# baseline (speedup 1.0000x reference)
"""Trainium2 Bass kernel for ContextQueryAttention (trilinear attention).

Math (per batch b; C:[D,N], Q:[D,M], W0:[3D], b0:[1]):
    S[n,m] = (Ct@w_c)[n] + (Qt@w_q)[m] + sum_d Ct[n,d]*w_qc[d]*Qt[m,d] + b0
    S_row = softmax_m(S), S_col = softmax_n(S)
    A  = S_row @ Qt                       # (N, D)
    Bt = S_row @ (S_col.T @ Ct)           # (N, D)  (N x N intermediate dropped)

Key restructurings (beyond the v1 algebra):
  * Bias folding INTO the matmul operands: the row-softmax only needs the
    q-score and the col-softmax only the c-score (other biases cancel), so
      X + cs = Ct @ (Q*w_qc + w_c)   and   X^T + qs = Qt @ (C*w_qc + w_q)
    i.e. one tensor_scalar (mul+add) per input makes every exp() biasless.
  * bf16 everywhere on the PE: full-rate 1 cycle/row at ANY moving size
    (f32r drops to 1/4 rate below 256-wide, which hit the (D+2)-wide col
    path), and half the SBUF/DMA traffic. f32 accumulation in PSUM.
  * Softmax denominators ride as ones-columns fused into the consuming
    matmuls; normalization is a per-partition reciprocal+scale of PSUM.
  * Ct / Qt come from DMA-engine transposes (dma_start_transpose, bf16),
    not PE transposes: zero PE/DVE/Act cost.
  * A|Bt are emitted as one [N, 2D+2] bf16 tensor (single big DMA per
    batch, 516B lines); host splits/upcasts. Inputs are pre-cast to bf16
    host-side (harness feeds f32; bf16 is within the accuracy budget).

Sharding: data-parallel over batch, 8 batches per core on 8 cores.
"""

import contextlib

import numpy as np

import concourse.bass as bass
import concourse.bacc as bacc
import concourse.tile as tile
from concourse import mybir
from concourse.bass_utils import run_bass_kernel_spmd

F32 = mybir.dt.float32
BF16 = mybir.dt.bfloat16

# Problem shape (hardcoded per spec)
B, D, N, M = 64, 128, 1024, 256
NCORES = 8
BPC = B // NCORES  # batches per core
NK = N // 128      # context chunks (8)
MJ = M // 128      # query chunks (2)
W = 2 * D + 2      # row-path width: [A | rowsum rowsum | Bt]


def build_kernel(bpc: int = BPC, repeats: int = 1, unroll: int = 1) -> bass.Bass:
    nc = bacc.Bacc("TRN2", target_bir_lowering=False, debug=False)

    CQ16 = nc.dram_tensor("CQ16", [bpc, D, N + M], BF16, kind="ExternalInput").ap()
    # host-transposed packed chunks: NK x [Ct|1 1] (130 cols) then
    # MJ x [Qt|1 1|G-slot] (258 cols); G written into its slot on device
    TQW = NK * (D + 2) + MJ * W
    TQ16 = nc.dram_tensor("TQ16", [bpc, 128, TQW], BF16, kind="ExternalInput").ap()
    # weight columns: [w_q | w_c | w_qc] as [D, 1] f32 scalars
    WQ = nc.dram_tensor("WQ", [D, 1], F32, kind="ExternalInput").ap()
    WC = nc.dram_tensor("WC", [D, 1], F32, kind="ExternalInput").ap()
    WQC = nc.dram_tensor("WQC", [D, 1], F32, kind="ExternalInput").ap()
    AB16 = nc.dram_tensor("AB16", [bpc, N, W], BF16, kind="ExternalOutput").ap()

    with tile.TileContext(nc) as tc:
        with (
            tc.tile_pool(name="singles", bufs=1) as singles,
            tc.tile_pool(name="inp", bufs=3) as pool_in,
            tc.tile_pool(name="sc", bufs=3) as pool_sc,
            tc.tile_pool(name="e", bufs=3) as pool_e,
            tc.tile_pool(name="tg", bufs=3) as pool_tg,
            tc.tile_pool(name="sm", bufs=3) as pool_sm,
            tc.tile_pool(name="out", bufs=3) as pool_out,
            tc.tile_pool(name="pp_x", bufs=2, space="PSUM") as pp_x,
            tc.tile_pool(name="pp_xt", bufs=1, space="PSUM") as pp_xt,
            tc.tile_pool(name="pp_g", bufs=2, space="PSUM") as pp_g,
            tc.tile_pool(name="pp_ab", bufs=2, space="PSUM") as pp_ab,
        ):
            wq = singles.tile([D, 1], F32)
            wc = singles.tile([D, 1], F32)
            wqc = singles.tile([D, 1], F32)
            nc.sync.dma_start(out=wq, in_=WQ)
            nc.sync.dma_start(out=wc, in_=WC)
            nc.sync.dma_start(out=wqc, in_=WQC)

            rep_ctx = (
                tc.For_i(
                    0,
                    repeats,
                    1,
                    hint_engines=(
                        mybir.EngineType.PE,
                        mybir.EngineType.DVE,
                        mybir.EngineType.Activation,
                        mybir.EngineType.SP,
                    ),
                )
                if repeats > 1
                else contextlib.nullcontext()
            )
            with rep_ctx:
                # Software pipeline, one-round lag per stage so every
                # engine's in-order queue only ever sees work whose inputs
                # are (nearly) ready:
                #   round r: load(r) | scores(r-1) | output(r-2)
                # PE order per round: col(r-2), X(r-1), XT(r-1), row(r-2)
                # puts the G-divide (DVE) latency under the score matmuls.
                tiles: dict[int, dict] = {}

                def stage_load(b):
                    t = {}
                    t["cq"] = pool_in.tile([D, N + M], BF16, tag="cq", name="cq")
                    nc.gpsimd.dma_start(out=t["cq"], in_=CQ16[b % bpc])
                    t["cb"] = t["cq"][:, 0:N]
                    t["qb"] = t["cq"][:, N : N + M]
                    ctq = pool_tg.tile([128, NK * (D + 2) + MJ * W], BF16, tag="ctq")
                    nc.sync.dma_start(out=ctq, in_=TQ16[b % bpc])
                    t["ctq"] = ctq
                    tiles[b] = t

                def stage_scores_pre(b):
                    # bias-folded scaled operands:
                    # qswc[d,m] = Q*w_qc + w_c -> Ct @ qswc = X + cs
                    # cswq[d,n] = C*w_qc + w_q -> Qt @ cswq = X^T + qs
                    t = tiles[b]
                    cb, qb = t["cb"], t["qb"]
                    qswc = pool_sc.tile([D, M], BF16, tag="qswc")
                    nc.vector.tensor_scalar(
                        out=qswc, in0=qb, scalar1=wqc, scalar2=wc,
                        op0=mybir.AluOpType.mult, op1=mybir.AluOpType.add,
                    )
                    cswq = pool_sc.tile([D, N], BF16, tag="cswq")
                    nc.vector.tensor_scalar(
                        out=cswq, in0=cb, scalar1=wqc, scalar2=wq,
                        op0=mybir.AluOpType.mult, op1=mybir.AluOpType.add,
                    )
                    t.update(qswc=qswc, cswq=cswq)

                def stage_scores_mm(b, part):
                    # part 0: e_col = exp(X + cs), two k-chunks per PSUM bank
                    # part 1: e_row = exp(X^T + qs), [m-part, j, n]
                    t = tiles[b]
                    cb, qb = t["cb"], t["qb"]
                    if part == 0:
                        e_col = pool_e.tile([128, NK, M], BF16, tag="e_col")
                        for k2 in range(NK // 2):
                            px = pp_x.tile([128, 2 * M], F32, tag="px")
                            for h in range(2):
                                k = 2 * k2 + h
                                nc.tensor.matmul(
                                    px[:, h * M : (h + 1) * M],
                                    cb[:, k * 128 : (k + 1) * 128],
                                    t["qswc"],
                                    start=True,
                                    stop=True,
                                )
                            nc.scalar.activation(
                                out=e_col[:, 2 * k2 : 2 * k2 + 2, :],
                                in_=px,
                                func=mybir.ActivationFunctionType.Exp,
                            )
                        t.update(e_col=e_col)
                    else:
                        e_row = pool_e.tile([128, MJ, N], BF16, tag="e_row")
                        for j in range(MJ):
                            qbj = qb[:, j * 128 : (j + 1) * 128]
                            pxt = pp_xt.tile([128, N], F32, tag="pxt")
                            for h in range(N // 512):
                                nc.tensor.matmul(
                                    pxt[:, h * 512 : (h + 1) * 512],
                                    qbj,
                                    t["cswq"][:, h * 512 : (h + 1) * 512],
                                    start=True,
                                    stop=True,
                                )
                            nc.scalar.activation(
                                out=e_row[:, j, :],
                                in_=pxt,
                                func=mybir.ActivationFunctionType.Exp,
                            )
                        t.update(e_row=e_row)

                def stage_out_col(b):
                    # col path: G_j = (E_col^T @ [Ct|1 1]) / colsum into qtg
                    t = tiles[b]
                    for j in range(MJ):
                        pg = pp_g.tile([128, D + 2], F32, tag="pg")
                        for k in range(NK):
                            nc.tensor.matmul(
                                pg,
                                t["e_col"][:, k, j * 128 : (j + 1) * 128],
                                t["ctq"][:, k * (D + 2) : (k + 1) * (D + 2)],
                                start=(k == 0),
                                stop=(k == NK - 1),
                            )
                        rcol = pool_sm.tile([128, 1], F32, tag=f"rcol{j}")
                        nc.vector.reciprocal(out=rcol, in_=pg[:, D : D + 1])
                        qoff = NK * (D + 2) + j * W
                        nc.vector.tensor_scalar_mul(
                            out=t["ctq"][:, qoff + D + 2 : qoff + W],
                            in0=pg[:, 0:D],
                            scalar1=rcol,
                        )

                def stage_out_row(b):
                    # row path: [A | rowsum rowsum | Bt] = E_row^T @ [Qt|1 1|G]
                    t = tiles.pop(b)
                    ab = pool_out.tile([128, NK, W], BF16, tag="ab")
                    for k in range(NK):
                        pab = pp_ab.tile([128, W], F32, tag="pab")
                        for j in range(MJ):
                            nc.tensor.matmul(
                                pab,
                                t["e_row"][:, j, k * 128 : (k + 1) * 128],
                                t["ctq"][
                                    :,
                                    NK * (D + 2) + j * W : NK * (D + 2) + (j + 1) * W,
                                ],
                                start=(j == 0),
                                stop=(j == MJ - 1),
                            )
                        rrow = pool_sm.tile([128, 1], F32, tag=f"rrow{k}")
                        nc.vector.reciprocal(out=rrow, in_=pab[:, D : D + 1])
                        nc.vector.tensor_scalar_mul(
                            out=ab[:, k, :], in0=pab, scalar1=rrow
                        )
                    nc.gpsimd.dma_start(
                        out=AB16[b % bpc].rearrange("(k p) c -> p k c", p=128), in_=ab
                    )

                stage_fns = {
                    "L": stage_load,
                    "P": stage_scores_pre,
                    "C": stage_out_col,
                    "M": lambda b: stage_scores_mm(b, 0),
                    "m": lambda b: stage_scores_mm(b, 1),
                    "R": stage_out_row,
                }
                lag = {"L": 0, "P": 1, "M": 1, "m": 1, "C": 2, "R": 2}
                nvb = bpc * unroll
                for r in range(nvb + 2):
                    for s in "LPCMmR":
                        bb = r - lag[s]
                        if 0 <= bb < nvb:
                            stage_fns[s](bb)
    nc.finalize()
    return nc


def make_in_maps(C, Q, W0, bpc: int = BPC, ncores: int = NCORES):
    """Host-side staging: cast to bf16, slice per core, split W0 columns."""
    import ml_dtypes

    bf = ml_dtypes.bfloat16
    C = np.asarray(C, np.float32)
    Q = np.asarray(Q, np.float32)
    CQ = np.concatenate([C, Q], axis=2)
    CQ16 = np.ascontiguousarray(CQ).astype(bf)
    # packed transposed chunks: NK x [Ct|1 1] then MJ x [Qt|1 1|0(G)]
    Bfull = C.shape[0]
    tp = CQ.transpose(0, 2, 1).reshape(Bfull, NK + MJ, 128, D).transpose(0, 2, 1, 3)
    tq = np.zeros((Bfull, 128, NK * (D + 2) + MJ * W), np.float32)
    for k in range(NK):
        tq[:, :, k * (D + 2) : k * (D + 2) + D] = tp[:, :, k]
        tq[:, :, k * (D + 2) + D : (k + 1) * (D + 2)] = 1.0
    for j in range(MJ):
        qoff = NK * (D + 2) + j * W
        tq[:, :, qoff : qoff + D] = tp[:, :, NK + j]
        tq[:, :, qoff + D : qoff + D + 2] = 1.0
    TQ16 = tq.astype(bf)
    W0 = np.asarray(W0, np.float32)
    wq = np.ascontiguousarray(W0[0:D].reshape(D, 1))
    wc = np.ascontiguousarray(W0[D : 2 * D].reshape(D, 1))
    wqc = np.ascontiguousarray(W0[2 * D : 3 * D].reshape(D, 1))
    return [
        {
            "CQ16": CQ16[i * bpc : (i + 1) * bpc],
            "TQ16": TQ16[i * bpc : (i + 1) * bpc],
            "WQ": wq,
            "WC": wc,
            "WQC": wqc,
        }
        for i in range(ncores)
    ]


_NC_CACHE = None


def kernel(C, Q, W0, b0):
    global _NC_CACHE
    if _NC_CACHE is None:
        _NC_CACHE = build_kernel()
    nc = _NC_CACHE

    in_maps = make_in_maps(C, Q, W0)
    res = run_bass_kernel_spmd(nc, in_maps, core_ids=list(range(NCORES)))
    ab = np.concatenate(
        [np.asarray(res.results[i]["AB16"]) for i in range(NCORES)], axis=0
    )
    ab = ab.astype(np.float32)
    A = np.ascontiguousarray(ab[:, :, 0:D])
    Bt = np.ascontiguousarray(ab[:, :, D + 2 : W])
    return (A, Bt)


# revision 21
# speedup vs baseline: 1.1331x; 1.1331x over previous
"""Trainium2 Bass kernel for ContextQueryAttention (trilinear attention).

Math (per batch b; C:[D,N], Q:[D,M], W0:[3D], b0:[1]):
    S[n,m] = (Ct@w_c)[n] + (Qt@w_q)[m] + sum_d Ct[n,d]*w_qc[d]*Qt[m,d] + b0
    S_row = softmax_m(S), S_col = softmax_n(S)
    A  = S_row @ Qt                       # (N, D)
    Bt = S_row @ (S_col.T @ Ct)           # (N, D)  (N x N intermediate dropped)

Key restructurings (beyond the v1 algebra):
  * Bias folding INTO the matmul operands: the row-softmax only needs the
    q-score and the col-softmax only the c-score (other biases cancel), so
      X + cs = Ct @ (Q*w_qc + w_c)   and   X^T + qs = Qt @ (C*w_qc + w_q)
    i.e. one tensor_scalar (mul+add) per input makes every exp() biasless.
  * bf16 everywhere on the PE: full-rate 1 cycle/row at ANY moving size
    (f32r drops to 1/4 rate below 256-wide, which hit the (D+2)-wide col
    path), and half the SBUF/DMA traffic. f32 accumulation in PSUM.
  * Softmax denominators ride as ones-columns fused into the consuming
    matmuls; normalization is a per-partition reciprocal+scale of PSUM.
  * Ct / Qt come from DMA-engine transposes (dma_start_transpose, bf16),
    not PE transposes: zero PE/DVE/Act cost.
  * A|Bt are emitted as one [N, 2D+2] bf16 tensor (single big DMA per
    batch, 516B lines); host splits/upcasts. Inputs are pre-cast to bf16
    host-side (harness feeds f32; bf16 is within the accuracy budget).

Sharding: data-parallel over batch, 8 batches per core on 8 cores.
"""

import contextlib

import numpy as np

import concourse.bass as bass
import concourse.bacc as bacc
import concourse.tile as tile
from concourse import mybir
from concourse.bass_utils import run_bass_kernel_spmd

F32 = mybir.dt.float32
BF16 = mybir.dt.bfloat16

# Problem shape (hardcoded per spec)
B, D, N, M = 64, 128, 1024, 256
NCORES = 8
BPC = B // NCORES  # batches per core
NK = N // 128      # context chunks (8)
MJ = M // 128      # query chunks (2)
W = 2 * D + 2      # row-path width: [A | rowsum rowsum | Bt]


def build_kernel(bpc: int = BPC, repeats: int = 1, unroll: int = 1) -> bass.Bass:
    nc = bacc.Bacc("TRN2", target_bir_lowering=False, debug=False)

    CQ16 = nc.dram_tensor("CQ16", [bpc, D, N + M], BF16, kind="ExternalInput").ap()
    # host-transposed packed chunks: NK x [Ct|1 1] (130 cols) then
    # MJ x [Qt|1 1|G-slot] (258 cols); G written into its slot on device
    TQW = NK * (D + 2) + MJ * W
    TQ16 = nc.dram_tensor("TQ16", [bpc, 128, TQW], BF16, kind="ExternalInput").ap()
    # weight columns: [w_q | w_c | w_qc] as [D, 1] f32 scalars
    WQ = nc.dram_tensor("WQ", [D, 1], F32, kind="ExternalInput").ap()
    WC = nc.dram_tensor("WC", [D, 1], F32, kind="ExternalInput").ap()
    WQC = nc.dram_tensor("WQC", [D, 1], F32, kind="ExternalInput").ap()
    AB16 = nc.dram_tensor("AB16", [bpc, N, W], BF16, kind="ExternalOutput").ap()

    with tile.TileContext(nc) as tc:
        with (
            tc.tile_pool(name="singles", bufs=1) as singles,
            tc.tile_pool(name="inp", bufs=3) as pool_in,
            tc.tile_pool(name="sc", bufs=3) as pool_sc,
            tc.tile_pool(name="e", bufs=3) as pool_e,
            tc.tile_pool(name="tg", bufs=3) as pool_tg,
            tc.tile_pool(name="sm", bufs=3) as pool_sm,
            tc.tile_pool(name="out", bufs=3) as pool_out,
            tc.tile_pool(name="pp_x", bufs=2, space="PSUM") as pp_x,
            tc.tile_pool(name="pp_xt", bufs=2, space="PSUM") as pp_xt,
            tc.tile_pool(name="pp_g", bufs=2, space="PSUM") as pp_g,
            tc.tile_pool(name="pp_ab", bufs=2, space="PSUM") as pp_ab,
        ):
            wq = singles.tile([D, 1], F32)
            wc = singles.tile([D, 1], F32)
            wqc = singles.tile([D, 1], F32)
            nc.sync.dma_start(out=wq, in_=WQ)
            nc.sync.dma_start(out=wc, in_=WC)
            nc.sync.dma_start(out=wqc, in_=WQC)

            rep_ctx = (
                tc.For_i(
                    0,
                    repeats,
                    1,
                    hint_engines=(
                        mybir.EngineType.PE,
                        mybir.EngineType.DVE,
                        mybir.EngineType.Activation,
                        mybir.EngineType.SP,
                    ),
                )
                if repeats > 1
                else contextlib.nullcontext()
            )
            with rep_ctx:
                # Software pipeline, one-round lag per stage so every
                # engine's in-order queue only ever sees work whose inputs
                # are (nearly) ready:
                #   round r: load(r) | scores(r-1) | output(r-2)
                # PE order per round: col(r-2), X(r-1), XT(r-1), row(r-2)
                # puts the G-divide (DVE) latency under the score matmuls.
                tiles: dict[int, dict] = {}

                def stage_load(b):
                    t = {}
                    t["cq"] = pool_in.tile([D, N + M], BF16, tag="cq", name="cq")
                    nc.gpsimd.dma_start(out=t["cq"], in_=CQ16[b % bpc])
                    t["cb"] = t["cq"][:, 0:N]
                    t["qb"] = t["cq"][:, N : N + M]
                    ctq = pool_tg.tile([128, NK * (D + 2) + MJ * W], BF16, tag="ctq")
                    nc.sync.dma_start(out=ctq, in_=TQ16[b % bpc])
                    t["ctq"] = ctq
                    tiles[b] = t

                def stage_scores_pre(b):
                    # bias-folded scaled operands:
                    # qswc[d,m] = Q*w_qc + w_c -> Ct @ qswc = X + cs
                    # cswq[d,n] = C*w_qc + w_q -> Qt @ cswq = X^T + qs
                    t = tiles[b]
                    cb, qb = t["cb"], t["qb"]
                    qswc = pool_sc.tile([D, M], BF16, tag="qswc")
                    nc.vector.tensor_scalar(
                        out=qswc, in0=qb, scalar1=wqc, scalar2=wc,
                        op0=mybir.AluOpType.mult, op1=mybir.AluOpType.add,
                    )
                    cswq = pool_sc.tile([D, N], BF16, tag="cswq")
                    nc.vector.tensor_scalar(
                        out=cswq, in0=cb, scalar1=wqc, scalar2=wq,
                        op0=mybir.AluOpType.mult, op1=mybir.AluOpType.add,
                    )
                    t.update(qswc=qswc, cswq=cswq)

                def stage_scores_mm(b, part):
                    # part 0: e_col = exp(X + cs), two k-chunks per PSUM bank
                    # part 1: e_row = exp(X^T + qs), [m-part, j, n]
                    t = tiles[b]
                    cb, qb = t["cb"], t["qb"]
                    if part == 0:
                        e_col = pool_e.tile([128, NK, M], BF16, tag="e_col")
                        for k2 in range(NK // 2):
                            px = pp_x.tile([128, 2 * M], F32, tag="px")
                            for h in range(2):
                                k = 2 * k2 + h
                                nc.tensor.matmul(
                                    px[:, h * M : (h + 1) * M],
                                    cb[:, k * 128 : (k + 1) * 128],
                                    t["qswc"],
                                    start=True,
                                    stop=True,
                                )
                            nc.scalar.activation(
                                out=e_col[:, 2 * k2 : 2 * k2 + 2, :],
                                in_=px,
                                func=mybir.ActivationFunctionType.Exp,
                            )
                        t.update(e_col=e_col)
                    else:
                        e_row = pool_e.tile([128, MJ, N], BF16, tag="e_row")
                        for j in range(MJ):
                            qbj = qb[:, j * 128 : (j + 1) * 128]
                            for h in range(N // 512):
                                pxt = pp_xt.tile([128, 512], F32, tag="pxt")
                                nc.tensor.matmul(
                                    pxt,
                                    qbj,
                                    t["cswq"][:, h * 512 : (h + 1) * 512],
                                    start=True,
                                    stop=True,
                                )
                                nc.scalar.activation(
                                    out=e_row[:, j, h * 512 : (h + 1) * 512],
                                    in_=pxt,
                                    func=mybir.ActivationFunctionType.Exp,
                                )
                        t.update(e_row=e_row)

                def stage_out_col(b):
                    # col path: G_j = (E_col^T @ [Ct|1 1]) / colsum into qtg
                    t = tiles[b]
                    for j in range(MJ):
                        pg = pp_g.tile([128, D + 2], F32, tag="pg")
                        for k in range(NK):
                            nc.tensor.matmul(
                                pg,
                                t["e_col"][:, k, j * 128 : (j + 1) * 128],
                                t["ctq"][:, k * (D + 2) : (k + 1) * (D + 2)],
                                start=(k == 0),
                                stop=(k == NK - 1),
                            )
                        rcol = pool_sm.tile([128, 1], F32, tag=f"rcol{j}")
                        nc.vector.reciprocal(out=rcol, in_=pg[:, D : D + 1])
                        qoff = NK * (D + 2) + j * W
                        nc.vector.tensor_scalar_mul(
                            out=t["ctq"][:, qoff + D + 2 : qoff + W],
                            in0=pg[:, 0:D],
                            scalar1=rcol,
                        )

                def stage_out_row(b):
                    # row path: [A | rowsum rowsum | Bt] = E_row^T @ [Qt|1 1|G]
                    t = tiles.pop(b)
                    ab = pool_out.tile([128, NK, W], BF16, tag="ab")
                    for k in range(NK):
                        pab = pp_ab.tile([128, W], F32, tag="pab")
                        for j in range(MJ):
                            nc.tensor.matmul(
                                pab,
                                t["e_row"][:, j, k * 128 : (k + 1) * 128],
                                t["ctq"][
                                    :,
                                    NK * (D + 2) + j * W : NK * (D + 2) + (j + 1) * W,
                                ],
                                start=(j == 0),
                                stop=(j == MJ - 1),
                            )
                        rrow = pool_sm.tile([128, 1], F32, tag=f"rrow{k}")
                        nc.vector.reciprocal(out=rrow, in_=pab[:, D : D + 1])
                        nc.vector.tensor_scalar_mul(
                            out=ab[:, k, :], in0=pab, scalar1=rrow
                        )
                    nc.gpsimd.dma_start(
                        out=AB16[b % bpc].rearrange("(k p) c -> p k c", p=128), in_=ab
                    )

                stage_fns = {
                    "L": stage_load,
                    "P": stage_scores_pre,
                    "C": stage_out_col,
                    "M": lambda b: stage_scores_mm(b, 0),
                    "m": lambda b: stage_scores_mm(b, 1),
                    "R": stage_out_row,
                }
                lag = {"L": 0, "P": 1, "M": 1, "m": 1, "C": 2, "R": 2}
                nvb = bpc * unroll
                for r in range(nvb + 2):
                    for s in "LPCMmR":
                        bb = r - lag[s]
                        if 0 <= bb < nvb:
                            stage_fns[s](bb)
    nc.finalize()
    return nc


def make_in_maps(C, Q, W0, bpc: int = BPC, ncores: int = NCORES):
    """Host-side staging: cast to bf16, slice per core, split W0 columns."""
    import ml_dtypes

    bf = ml_dtypes.bfloat16
    C = np.asarray(C, np.float32)
    Q = np.asarray(Q, np.float32)
    CQ = np.concatenate([C, Q], axis=2)
    CQ16 = np.ascontiguousarray(CQ).astype(bf)
    # packed transposed chunks: NK x [Ct|1 1] then MJ x [Qt|1 1|0(G)]
    Bfull = C.shape[0]
    tp = CQ.transpose(0, 2, 1).reshape(Bfull, NK + MJ, 128, D).transpose(0, 2, 1, 3)
    tq = np.zeros((Bfull, 128, NK * (D + 2) + MJ * W), np.float32)
    for k in range(NK):
        tq[:, :, k * (D + 2) : k * (D + 2) + D] = tp[:, :, k]
        tq[:, :, k * (D + 2) + D : (k + 1) * (D + 2)] = 1.0
    for j in range(MJ):
        qoff = NK * (D + 2) + j * W
        tq[:, :, qoff : qoff + D] = tp[:, :, NK + j]
        tq[:, :, qoff + D : qoff + D + 2] = 1.0
    TQ16 = tq.astype(bf)
    W0 = np.asarray(W0, np.float32)
    wq = np.ascontiguousarray(W0[0:D].reshape(D, 1))
    wc = np.ascontiguousarray(W0[D : 2 * D].reshape(D, 1))
    wqc = np.ascontiguousarray(W0[2 * D : 3 * D].reshape(D, 1))
    return [
        {
            "CQ16": CQ16[i * bpc : (i + 1) * bpc],
            "TQ16": TQ16[i * bpc : (i + 1) * bpc],
            "WQ": wq,
            "WC": wc,
            "WQC": wqc,
        }
        for i in range(ncores)
    ]


_NC_CACHE = None


def kernel(C, Q, W0, b0):
    global _NC_CACHE
    if _NC_CACHE is None:
        _NC_CACHE = build_kernel()
    nc = _NC_CACHE

    in_maps = make_in_maps(C, Q, W0)
    res = run_bass_kernel_spmd(nc, in_maps, core_ids=list(range(NCORES)))
    ab = np.concatenate(
        [np.asarray(res.results[i]["AB16"]) for i in range(NCORES)], axis=0
    )
    ab = ab.astype(np.float32)
    A = np.ascontiguousarray(ab[:, :, 0:D])
    Bt = np.ascontiguousarray(ab[:, :, D + 2 : W])
    return (A, Bt)
